# revision 6
# baseline (speedup 1.0000x reference)
"""Trainium2 Bass kernel for grouped vector attention (sparse_attention).

Reference computation (B=2, L1=L2=512, D=256, g=16, n=16):
    Q = x_target @ Wq.T ; K = x_source @ Wk.T ; V = x_source @ Wv.T
    diff = Q.reshape(B,L1,1,n,g) - K.reshape(B,1,L2,n,g)
    scores = relu(einsum('bijng,g->bijn', relu(diff), w_mlp) + b_mlp)
    att = softmax(scores, axis=2)                      # over L2
    out = einsum('bijn,bjgn->bign', att, V.reshape(B,L2,g,n)).reshape(B,L1,D)

Sharding: 8 cores = 2 batches x 4 L1-quarters; 128 queries per core.

Per-core data layout:
  - Elementwise relu(Q-K) runs with d (channel) on partitions, j free:
      ScalarE:  tmp = relu(-K + Q_bias)            (one fused activation)
      VectorE:  tmp = min(K - Q, 0) = -relu(Q-K)   (one dual-op tensor_scalar,
                bf16; sign folded into that engine's sel matmul weights)
  - Grouped weighted sum over g=16 via TensorE matmul with a block-diagonal
    [128 x 32] sel matrix (w_mlp folded in; 16 real cols + 16 zero cols).
    Each query's 16 scores land in a 32-aligned PSUM partition slot
    (4 queries per [128, 512] PSUM half; two halves = one [128,1024] tile).
  - p = exp(scores + b) off PSUM (bias baked); pc = max(p, 1) on VectorE.
  - pc transposed via DMA-transpose (128x128 bf16) into (j, query-col) tiles;
    att@V plus a ones-column (softmax denominator) as TensorE matmuls with a
    strided lhsT AP that skips the 16 garbage columns per 32-slot.
  - Group extraction via mask-multiply (normalization fused) + grouped reduce.
  - Host does the final (par,q,nn,gg) -> (i, d) permutation and assembly.
"""

import numpy as np

import concourse.bass as bass
import concourse.bacc as bacc
import concourse.tile as tile
import concourse.mybir as mybir
from concourse.bass_utils import run_bass_kernel_spmd

import ml_dtypes

F32 = mybir.dt.float32
BF16 = mybir.dt.bfloat16
AL = mybir.AluOpType
AF = mybir.ActivationFunctionType

B, L1, L2, D = 2, 512, 512, 256
G = 16           # group size (d_group)
N = 16           # number of groups
NCORES = 8
ISH = 128        # queries per core (L1 / 4)
NPAIR = 16       # 16 pairs; each pair = 8 queries (2 halves x 4 queries)
BF = ml_dtypes.bfloat16

# queries (by slot q in 0..3) whose relu(Q-K) runs on ScalarE; rest VectorE.
ACT_SLOTS = (3,)


def _build(b_val: float):
    """Build + compile the per-core Bass graph. Same graph for all 8 cores."""
    nc = bacc.Bacc(
        "TRN2", target_bir_lowering=False, debug=False, enable_asserts=False
    )

    # ---- DRAM parameters (per-core shards, host-prepped) ----
    xtT_d = nc.dram_tensor("xtT", [2, 128, ISH], BF16, kind="ExternalInput")
    xsT_d = nc.dram_tensor("xsT", [2, 128, L2], BF16, kind="ExternalInput")
    wqT_d = nc.dram_tensor("wqT", [2, 128, D], BF16, kind="ExternalInput")
    wkT_d = nc.dram_tensor("wkT", [2, 128, D], BF16, kind="ExternalInput")
    wvT_d = nc.dram_tensor("wvT", [2, 128, D], BF16, kind="ExternalInput")
    sel_d = nc.dram_tensor("sel", [2, 2, 128, 32], BF16, kind="ExternalInput")
    mask_d = nc.dram_tensor("mask", [128, D], BF16, kind="ExternalInput")
    out_d = nc.dram_tensor("out", [NPAIR, 128, N], F32, kind="ExternalOutput")

    with tile.TileContext(nc) as tc:
        with (
            tc.tile_pool(name="const", bufs=1) as cpool,
            tc.tile_pool(name="work", bufs=3) as wpool,
            tc.tile_pool(name="tmps", bufs=6) as tpool,
            tc.tile_pool(name="ptp", bufs=3) as ppool,
            tc.tile_pool(name="ps_s", bufs=2, space="PSUM") as ps_pool,
            tc.tile_pool(name="ps_o", bufs=2, space="PSUM") as po_pool,
        ):
            # ---- load constants / inputs ----
            xtT = [cpool.tile([128, ISH], BF16, name=f"xtT{h}") for h in range(2)]
            xsT = [cpool.tile([128, L2], BF16, name=f"xsT{h}") for h in range(2)]
            wqT = [cpool.tile([128, D], BF16, name=f"wqT{h}") for h in range(2)]
            wkT = [cpool.tile([128, D], BF16, name=f"wkT{h}") for h in range(2)]
            wvT = [cpool.tile([128, D], BF16, name=f"wvT{h}") for h in range(2)]
            sel = [
                [cpool.tile([128, 32], BF16, name=f"sel{s}{h}") for h in range(2)]
                for s in range(2)
            ]
            mask = cpool.tile([128, D], BF16, name="mask")
            bml = cpool.tile([128, 1], F32, name="bml")
            nc.vector.memset(bml[:], float(b_val))
            for h in range(2):
                nc.sync.dma_start(xtT[h][:], xtT_d[h])
                nc.sync.dma_start(xsT[h][:], xsT_d[h])
                nc.sync.dma_start(wqT[h][:], wqT_d[h])
                nc.sync.dma_start(wkT[h][:], wkT_d[h])
                nc.sync.dma_start(wvT[h][:], wvT_d[h])
                for s in range(2):
                    nc.sync.dma_start(sel[s][h][:], sel_d[s, h])
            nc.sync.dma_start(mask[:], mask_d[:])

            # ---- projections on device ----
            # QT[eh] (128 e, 128 i) fp32 (used as per-partition scalar APs)
            QT = [cpool.tile([128, ISH], F32, name=f"QT{h}") for h in range(2)]
            for eh in range(2):
                psq = po_pool.tile([128, ISH], F32, name="psq", tag="ps_o")
                for dh in range(2):
                    nc.tensor.matmul(
                        psq[:],
                        wqT[dh][:, eh * 128 : (eh + 1) * 128],
                        xtT[dh][:],
                        start=(dh == 0),
                        stop=(dh == 1),
                    )
                nc.scalar.copy(QT[eh][:], psq[:])
            # KT[eh] (128 e, 512 j) bf16
            KT = [cpool.tile([128, L2], BF16, name=f"KT{h}") for h in range(2)]
            for eh in range(2):
                psk = ps_pool.tile([128, L2], F32, name="psk", tag="ps_s")
                for dh in range(2):
                    nc.tensor.matmul(
                        psk[:],
                        wkT[dh][:, eh * 128 : (eh + 1) * 128],
                        xsT[dh][:],
                        start=(dh == 0),
                        stop=(dh == 1),
                    )
                nc.vector.tensor_copy(KT[eh][:], psk[:])
            # V[jc] (128 j, 257 e) bf16 -- col 256 is the all-ones column
            V = [cpool.tile([128, D + 1], BF16, name=f"V{j}") for j in range(4)]
            for jc in range(4):
                psv = po_pool.tile([128, D], F32, name="psv", tag="ps_o")
                for dh in range(2):
                    nc.tensor.matmul(
                        psv[:],
                        xsT[dh][:, jc * 128 : (jc + 1) * 128],
                        wvT[dh][:],
                        start=(dh == 0),
                        stop=(dh == 1),
                    )
                nc.vector.tensor_copy(V[jc][:, 0:D], psv[:])
                nc.vector.memset(V[jc][:, D : D + 1], 1.0)

            # ---- main loop: 16 pairs x (2 halves x 4 queries) ----
            for pair in range(NPAIR):
                # scores: queries (pair, par, q) -> ps[32q:32q+32, 512par:+512]
                ps = ps_pool.tile([128, 2 * L2], F32, name="ps", tag="ps_s")
                for par in range(2):
                    for q in range(4):
                        i = 8 * pair + 4 * par + q
                        use_act = q in ACT_SLOTS
                        for h in range(2):
                            t = tpool.tile([128, L2], BF16, name="t", tag="t")
                            if use_act:
                                # t = relu(Q - K)
                                nc.scalar.activation(
                                    t[:],
                                    KT[h][:],
                                    AF.Relu,
                                    bias=QT[h][:, i : i + 1],
                                    scale=-1.0,
                                )
                                s_idx = 0
                            else:
                                # t = min(K - Q, 0) = -relu(Q - K)
                                nc.vector.tensor_scalar(
                                    t[:],
                                    KT[h][:],
                                    QT[h][:, i : i + 1],
                                    0.0,
                                    AL.subtract,
                                    AL.min,
                                )
                                s_idx = 1
                            nc.tensor.matmul(
                                ps[
                                    32 * q : 32 * q + 32,
                                    512 * par : 512 * par + 512,
                                ],
                                sel[s_idx][h][:],
                                t[:],
                                start=(h == 0),
                                stop=(h == 1),
                                tile_position=(0, 32 * q),
                            )
                # p = exp(scores + b); pc = max(p, 1)
                p = wpool.tile([128, 2 * L2], BF16, name="p", tag="p")
                nc.scalar.activation(p[:], ps[:], AF.Exp, bias=bml[:], scale=1.0)
                pc = wpool.tile([128, 2 * L2], BF16, name="pc", tag="pc")
                nc.vector.tensor_scalar(pc[:], p[:], 1.0, None, AL.max)
                # transpose pc into pt[jc] (128 j, 256 = (par, q, nn-slot))
                pt = []
                for jc in range(4):
                    t2 = ppool.tile([128, 256], BF16, name="t2", tag="pt")
                    for par in range(2):
                        nc.sync.dma_start_transpose(
                            t2[:, 128 * par : 128 * par + 128],
                            pc[:, 512 * par + 128 * jc : 512 * par + 128 * jc + 128],
                        )
                    pt.append(t2)
                # compact out the 16 garbage cols per 32-slot (matmul lhsT
                # must have a single free dim), then att @ [V | 1]:
                # po[m, :] for m = 64*par + 16*q + nn (dense)
                po = po_pool.tile([128, D + 1], F32, name="po", tag="ps_o")
                for jc in range(4):
                    pd = ppool.tile([128, 128], BF16, name="pd", tag="pd")
                    nc.vector.tensor_copy(
                        pd[:],
                        pt[jc][:].rearrange("j (s nn) -> j s nn", nn=32)[
                            :, :, 0:16
                        ],
                    )
                    nc.tensor.matmul(
                        po[:], pd[:], V[jc][:], start=(jc == 0), stop=(jc == 3)
                    )
                # normalize + mask + grouped reduce
                S = po[:, D : D + 1]
                rS = wpool.tile([128, 1], F32, name="rS", tag="rS")
                nc.vector.reciprocal(rS[:], S)
                mm = wpool.tile([128, D], BF16, name="mm", tag="mm")
                nc.vector.scalar_tensor_tensor(
                    mm[:], po[:, 0:D], rS[:], mask[:], AL.mult, AL.mult
                )
                y = wpool.tile([128, N], F32, name="y", tag="y")
                nc.vector.tensor_reduce(
                    y[:],
                    mm[:].rearrange("p (g x) -> p g x", x=G),
                    axis=mybir.AxisListType.X,
                    op=AL.add,
                )
                nc.sync.dma_start(out_d[pair], y[:])

    nc.compile()
    return nc


_CACHE: dict = {}


def _get_graph(b_val: float):
    key = round(float(b_val), 10)
    if key not in _CACHE:
        _CACHE[key] = _build(float(b_val))
    return _CACHE[key]


def _host_prep(x_source, x_target, Wq, Wk, Wv, w_mlp):
    """Build per-core input maps (numpy, bf16)."""
    w_full = np.tile(np.asarray(w_mlp, np.float32), D // G)  # w_full[d] = w[d%16]
    sel = np.zeros((2, 2, 128, 32), np.float32)
    for h in range(2):
        for dl in range(128):
            d = 128 * h + dl
            sel[0, h, dl, (d // G) % 8 + 8 * h] = w_full[d]
    sel[1] = -sel[0]
    # score for group n of query lives at psum row 32*q + n' where the matmul
    # M-window covers rows [32q, 32q+32) = sel cols [0, 32). Global n in 0..15:
    # half h contributes n in [8h, 8h+8) -> col (n - 8h) + 8h = n. So col n.
    mask = np.zeros((128, D), np.float32)
    for p in range(128):
        mask[p, p % G :: G] = 1.0

    def split_h(a):  # (256, X) -> (2, 128, X)
        return np.ascontiguousarray(a.reshape(2, 128, a.shape[1]))

    wq_b = split_h(np.asarray(Wq, np.float32).T).astype(BF)
    wk_b = split_h(np.asarray(Wk, np.float32).T).astype(BF)
    wv_b = split_h(np.asarray(Wv, np.float32).T).astype(BF)
    sel_b = sel.astype(BF)
    mask_b = mask.astype(BF)

    xsT = [
        split_h(np.asarray(x_source[b], np.float32).T).astype(BF) for b in range(B)
    ]
    in_maps = []
    for core in range(NCORES):
        b, qq = divmod(core, 4)
        i0 = qq * ISH
        xtT = split_h(np.asarray(x_target[b, i0 : i0 + ISH], np.float32).T).astype(
            BF
        )
        in_maps.append(
            {
                "xtT": xtT,
                "xsT": xsT[b],
                "wqT": wq_b,
                "wkT": wk_b,
                "wvT": wv_b,
                "sel": sel_b,
                "mask": mask_b,
            }
        )
    return in_maps


def _host_gather(results):
    """results[core]["out"] (NPAIR, 128, N) -> full (B, L1, D) output."""
    out = np.empty((B, L1, D), np.float32)
    for core in range(NCORES):
        b, qq = divmod(core, 4)
        i0 = qq * ISH
        y = results[core]["out"]  # (pair, p = 64*par + 16*q + nn, gg)
        y5 = y.reshape(NPAIR, 2, 4, N, G)  # (pair, par, q, nn, gg)
        # i = 8*pair + 4*par + q ; d = 16*gg + nn
        o = y5.transpose(0, 1, 2, 4, 3).reshape(ISH, D)
        out[b, i0 : i0 + ISH] = o.astype(np.float32)
    return out


def run(inputs, trace=False, **kwargs):
    nc = _get_graph(float(np.asarray(inputs["b_mlp"]).reshape(-1)[0]))
    in_maps = _host_prep(
        inputs["x_source"],
        inputs["x_target"],
        inputs["Wq"],
        inputs["Wk"],
        inputs["Wv"],
        inputs["w_mlp"],
    )
    res = run_bass_kernel_spmd(
        nc, in_maps, core_ids=list(range(NCORES)), trace=trace, **kwargs
    )
    return _host_gather(res.results), res


def kernel(**inputs) -> np.ndarray:
    out, _ = run(inputs, trace=False)
    return out


# revision 12
# speedup vs baseline: 2.2919x; 2.2919x over previous
"""Trainium2 Bass kernel for grouped vector attention (sparse_attention).

Reference computation (B=2, L1=L2=512, D=256, g=16, n=16):
    Q = x_target @ Wq.T ; K = x_source @ Wk.T ; V = x_source @ Wv.T
    diff = Q.reshape(B,L1,1,n,g) - K.reshape(B,1,L2,n,g)
    scores = relu(einsum('bijng,g->bijn', relu(diff), w_mlp) + b_mlp)
    att = softmax(scores, axis=2)                      # over L2
    out = einsum('bijn,bjgn->bign', att, V.reshape(B,L2,g,n)).reshape(B,L1,D)

Sharding: 8 cores = 2 batches x 4 L2(j)-quarters. Each core handles all 512
queries against its 128 source positions and produces partial (unnormalized)
outputs + partial softmax denominators; the host sums the 4 partials per
batch and divides. Sharding over j (not i) means the exp'd scores come out
with j on partitions — exactly what the att@V contraction needs, so there is
no on-chip transpose anywhere.

Per-core pipeline, for each quad of 4 source positions (32 quads):
  - tmp[d, i] = relu(Q[i,d] - K[j,d]) with d on partitions, i free:
      ScalarE:  activation(Relu, in=QT, bias=-K[:,j], scale=1)
      VectorE:  tensor_scalar(in=QT, s1=-K[:,j], s2=0, op0=add, op1=max)
  - grouped weighted sum over g=16 via TensorE matmul with block-diagonal
    [128 x 32] sel (w_mlp folded); j's 16 scores land in PSUM slot 32*jj.
  - p = exp(scores + b) off PSUM; pc = max(p, 1)   (= exp(relu(scores+b)))
  - V_sel[g][32*jj + nn, e] = V[4g+jj, e] * (e % 16 == nn)  (built once by a
    broadcast-DMA from a DRAM copy of V + one masked multiply)
  - out_partial[e, i]  += V_sel[g][:, e-half].T @ pc   (PSUM accumulation
    across all 32 quads);  S_partial[nn, i] += ones_sel.T @ pc
"""

import numpy as np

import concourse.bass as bass
import concourse.bacc as bacc
import concourse.tile as tile
import concourse.mybir as mybir
from concourse.bass_utils import run_bass_kernel_spmd

import ml_dtypes

F32 = mybir.dt.float32
BF16 = mybir.dt.bfloat16
AL = mybir.AluOpType
AF = mybir.ActivationFunctionType

B, L1, L2, D = 2, 512, 512, 256
G = 16           # group size (d_group)
N = 16           # number of groups
NCORES = 8
JSH = 128        # source positions per core (L2 / 4)
NQUAD = 32       # 32 quads of 4 source positions
BF = ml_dtypes.bfloat16

# fraction of (j,h) elementwise units on ScalarE (rest VectorE).
ACT_FRAC = 0.30


def _build(b_val: float):
    """Build + compile the per-core Bass graph. Same graph for all 8 cores."""
    nc = bacc.Bacc(
        "TRN2", target_bir_lowering=False, debug=False, enable_asserts=False
    )

    # ---- DRAM parameters (per-core shards, host-prepped) ----
    xtT_d = nc.dram_tensor("xtT", [2, 128, L1], BF16, kind="ExternalInput")
    xssT_d = nc.dram_tensor("xssT", [2, 128, JSH], BF16, kind="ExternalInput")
    wqT_d = nc.dram_tensor("wqT", [2, 128, D], BF16, kind="ExternalInput")
    wkT_d = nc.dram_tensor("wkT", [2, 128, D], BF16, kind="ExternalInput")
    wvT_d = nc.dram_tensor("wvT", [2, 128, D], BF16, kind="ExternalInput")
    sel_d = nc.dram_tensor("sel", [2, 128, 32], BF16, kind="ExternalInput")
    vmask_d = nc.dram_tensor("vmask", [128, D], BF16, kind="ExternalInput")
    ones_d = nc.dram_tensor("ones_sel", [128, N], BF16, kind="ExternalInput")
    outp_d = nc.dram_tensor("outp", [2, 128, L1], F32, kind="ExternalOutput")
    souts_d = nc.dram_tensor("souts", [N, L1], F32, kind="ExternalOutput")
    vdram = nc.dram_tensor("vdram", [JSH, D], BF16)

    # elementwise engine assignment per (quad, jj): ~ACT_FRAC on ScalarE
    n_units = NQUAD * 4
    use_act_tab = [
        (int(u * ACT_FRAC) != int((u + 1) * ACT_FRAC)) for u in range(n_units)
    ]

    with tile.TileContext(nc) as tc:
        with (
            tc.tile_pool(name="const", bufs=1) as cpool,
            tc.tile_pool(name="vselp", bufs=1) as vpool,
            tc.tile_pool(name="work", bufs=3) as wpool,
            tc.tile_pool(name="vbp", bufs=4) as vbpool,
            tc.tile_pool(name="tmps", bufs=6) as tpool,
            tc.tile_pool(name="ps_s", bufs=2, space="PSUM") as ps_pool,
            tc.tile_pool(name="ps_acc", bufs=1, space="PSUM") as pa_pool,
        ):
            # ---- load constants / inputs ----
            xtT = [cpool.tile([128, L1], BF16, name=f"xtT{h}") for h in range(2)]
            xssT = [cpool.tile([128, JSH], BF16, name=f"xssT{h}") for h in range(2)]
            wqT = [cpool.tile([128, D], BF16, name=f"wqT{h}") for h in range(2)]
            wkT = [cpool.tile([128, D], BF16, name=f"wkT{h}") for h in range(2)]
            wvT = [cpool.tile([128, D], BF16, name=f"wvT{h}") for h in range(2)]
            sel = [cpool.tile([128, 32], BF16, name=f"sel{h}") for h in range(2)]
            vmask = cpool.tile([128, D], BF16, name="vmask")
            ones_sel = cpool.tile([128, N], BF16, name="ones_sel")
            bml = cpool.tile([128, 1], F32, name="bml")
            nc.vector.memset(bml[:], float(b_val))
            for h in range(2):
                nc.sync.dma_start(xtT[h][:], xtT_d[h])
                nc.sync.dma_start(xssT[h][:], xssT_d[h])
                nc.scalar.dma_start(wqT[h][:], wqT_d[h])
                nc.scalar.dma_start(wkT[h][:], wkT_d[h])
                nc.scalar.dma_start(wvT[h][:], wvT_d[h])
                nc.scalar.dma_start(sel[h][:], sel_d[h])
            nc.sync.dma_start(vmask[:], vmask_d[:])
            nc.sync.dma_start(ones_sel[:], ones_d[:])

            # ---- projections on device ----
            # QT[h] (128 e, 512 i) bf16
            QT = [cpool.tile([128, L1], BF16, name=f"QT{h}") for h in range(2)]
            for eh in range(2):
                psq = ps_pool.tile([128, L1], F32, name="psq", tag="ps_s")
                for dh in range(2):
                    nc.tensor.matmul(
                        psq[:],
                        wqT[dh][:, eh * 128 : (eh + 1) * 128],
                        xtT[dh][:],
                        start=(dh == 0),
                        stop=(dh == 1),
                    )
                nc.vector.tensor_copy(QT[eh][:], psq[:])
            # KTn[h] (128 e, 128 j) f32, negated (bias / scalar operand)
            KTn = [cpool.tile([128, JSH], F32, name=f"KTn{h}") for h in range(2)]
            for eh in range(2):
                psk = ps_pool.tile([128, JSH], F32, name="psk", tag="psk")
                for dh in range(2):
                    nc.tensor.matmul(
                        psk[:],
                        wkT[dh][:, eh * 128 : (eh + 1) * 128],
                        xssT[dh][:],
                        start=(dh == 0),
                        stop=(dh == 1),
                    )
                nc.vector.tensor_scalar(KTn[eh][:], psk[:], -1.0, None, AL.mult)
            # V (128 j, 256 e) bf16 -> DRAM (for the broadcast reload)
            Vt = cpool.tile([128, D], BF16, name="Vt")
            psv = ps_pool.tile([128, D], F32, name="psv", tag="psk")
            for dh in range(2):
                nc.tensor.matmul(
                    psv[:],
                    xssT[dh][:],
                    wvT[dh][:],
                    start=(dh == 0),
                    stop=(dh == 1),
                )
            nc.vector.tensor_copy(Vt[:], psv[:])
            nc.sync.dma_start(vdram[:], Vt[:])

            # V_sel[g] (128, 256): row 32*jj+nn = V[4g+jj, :] * (e%16 == nn)
            V_sel = []
            for g in range(NQUAD):
                src = (
                    vdram.ap()[4 * g : 4 * g + 4, :]
                    .unsqueeze(1)
                    .broadcast_to((4, 32, D))
                )
                vs = vpool.tile([128, D], BF16, name=f"vs{g}")
                nc.sync.dma_start(vs[:], src)
                nc.vector.tensor_tensor(vs[:], vs[:], vmask[:], op=AL.mult)
                V_sel.append(vs)

            # ---- accumulators ----
            ops = [
                pa_pool.tile([128, L1], F32, name=f"ops{eh}") for eh in range(2)
            ]
            sps = pa_pool.tile([16, L1], F32, name="sps")

            # ---- main loop: 32 quads of 4 source positions ----
            for g in range(NQUAD):
                ps = ps_pool.tile([128, L1], F32, name="ps", tag="ps_s")
                for jj in range(4):
                    j = 4 * g + jj
                    use_act = use_act_tab[g * 4 + jj]
                    for h in range(2):
                        t = tpool.tile([128, L1], BF16, name="t", tag="t")
                        if use_act:
                            # t = relu(Q + (-K))
                            nc.scalar.activation(
                                t[:],
                                QT[h][:],
                                AF.Relu,
                                bias=KTn[h][:, j : j + 1],
                                scale=1.0,
                            )
                        else:
                            # t = max(Q + (-K), 0)
                            nc.vector.tensor_scalar(
                                t[:],
                                QT[h][:],
                                KTn[h][:, j : j + 1],
                                0.0,
                                AL.add,
                                AL.max,
                            )
                        nc.tensor.matmul(
                            ps[32 * jj : 32 * jj + 32, :],
                            sel[h][:],
                            t[:],
                            start=(h == 0),
                            stop=(h == 1),
                            tile_position=(0, 32 * jj),
                        )
                # p = exp(scores + b); pc = max(p, 1) = exp(relu(scores + b))
                p = wpool.tile([128, L1], BF16, name="p", tag="p")
                nc.scalar.activation(p[:], ps[:], AF.Exp, bias=bml[:], scale=1.0)
                pc = wpool.tile([128, L1], BF16, name="pc", tag="pc")
                nc.vector.tensor_scalar(pc[:], p[:], 1.0, None, AL.max)
                # accumulate partial outputs and denominators
                for eh in range(2):
                    nc.tensor.matmul(
                        ops[eh][:],
                        V_sel[g][:, eh * 128 : (eh + 1) * 128],
                        pc[:],
                        start=(g == 0),
                        stop=(g == NQUAD - 1),
                        skip_group_check=True,
                    )
                nc.tensor.matmul(
                    sps[:],
                    ones_sel[:, 0:N],
                    pc[:],
                    start=(g == 0),
                    stop=(g == NQUAD - 1),
                    skip_group_check=True,
                )

            # ---- evacuate + store ----
            for eh in range(2):
                ou = wpool.tile([128, L1], F32, name="ou", tag="ou")
                nc.vector.tensor_copy(ou[:], ops[eh][:])
                nc.sync.dma_start(outp_d[eh], ou[:])
            so = wpool.tile([16, L1], F32, name="so")
            nc.scalar.copy(so[:], sps[:])
            nc.scalar.dma_start(souts_d[:], so[:])

    nc.compile()
    return nc


_CACHE: dict = {}


def _get_graph(b_val: float):
    key = round(float(b_val), 10)
    if key not in _CACHE:
        _CACHE[key] = _build(float(b_val))
    return _CACHE[key]


def _host_prep(x_source, x_target, Wq, Wk, Wv, w_mlp):
    """Build per-core input maps (numpy, bf16)."""
    w_full = np.tile(np.asarray(w_mlp, np.float32), D // G)  # w_full[d] = w[d%16]
    sel = np.zeros((2, 128, 32), np.float32)
    for h in range(2):
        for dl in range(128):
            d = 128 * h + dl
            sel[h, dl, d // G] = w_full[d]
    # V_sel mask: row p = 32*jj + s (s<16 valid), col e: keep if e%16 == s
    vmask = np.zeros((128, D), np.float32)
    for p in range(128):
        s = p % 32
        if s < 16:
            vmask[p, s::G] = 1.0
    # S selector: row p = 32*jj + s -> column s (s < 16)
    ones_sel = np.zeros((128, N), np.float32)
    for p in range(128):
        s = p % 32
        if s < 16:
            ones_sel[p, s] = 1.0

    def split_h(a):  # (256, X) -> (2, 128, X)
        return np.ascontiguousarray(a.reshape(2, 128, a.shape[1]))

    wq_b = split_h(np.asarray(Wq, np.float32).T).astype(BF)
    wk_b = split_h(np.asarray(Wk, np.float32).T).astype(BF)
    wv_b = split_h(np.asarray(Wv, np.float32).T).astype(BF)
    sel_b = sel.astype(BF)
    vmask_b = vmask.astype(BF)
    ones_b = ones_sel.astype(BF)

    xtT = [
        split_h(np.asarray(x_target[b], np.float32).T).astype(BF) for b in range(B)
    ]
    xsT = [np.asarray(x_source[b], np.float32).T for b in range(B)]
    in_maps = []
    for core in range(NCORES):
        b, jq = divmod(core, 4)
        j0 = jq * JSH
        xssT = split_h(xsT[b][:, j0 : j0 + JSH]).astype(BF)
        in_maps.append(
            {
                "xtT": xtT[b],
                "xssT": xssT,
                "wqT": wq_b,
                "wkT": wk_b,
                "wvT": wv_b,
                "sel": sel_b,
                "vmask": vmask_b,
                "ones_sel": ones_b,
            }
        )
    return in_maps


def _host_gather(results):
    """Sum partials over j-shards, normalize, reshape to (B, L1, D)."""
    out = np.empty((B, L1, D), np.float32)
    for b in range(B):
        cores = [b * 4 + jq for jq in range(4)]
        U = sum(
            results[c]["outp"].reshape(D, L1).astype(np.float64) for c in cores
        )  # (e, i)
        S = sum(results[c]["souts"].astype(np.float64) for c in cores)  # (nn, i)
        att = U / S[np.arange(D) % N, :]  # (e, i)
        out[b] = att.T.astype(np.float32)
    return out


def run(inputs, trace=False, **kwargs):
    nc = _get_graph(float(np.asarray(inputs["b_mlp"]).reshape(-1)[0]))
    in_maps = _host_prep(
        inputs["x_source"],
        inputs["x_target"],
        inputs["Wq"],
        inputs["Wk"],
        inputs["Wv"],
        inputs["w_mlp"],
    )
    res = run_bass_kernel_spmd(
        nc, in_maps, core_ids=list(range(NCORES)), trace=trace, **kwargs
    )
    return _host_gather(res.results), res


def kernel(**inputs) -> np.ndarray:
    out, _ = run(inputs, trace=False)
    return out
